# revision 1
# baseline (speedup 1.0000x reference)
"""Trainium2 Bass kernel for nn_DeltaRecurrentUpdate.

Reference computation (per batch b, one-shot chunked delta-rule update):
    k   = hidden_states @ key_w + key_b            # [l, h]
    k   = k / max(||k||_row, 1e-12)                # L2 normalize rows
    v   = hidden_states @ value_w + value_b        # [l, h]
    v   = v - k @ prev_cache                       # [l, h]
    out = prev_cache + k^T @ v                     # [h, h]

Strategy: data-parallel over batch (B=8 == 8 NeuronCores, zero collectives).

Key algebraic restructurings (per core):
  1. Bias folded into the projections by augmenting hs with a ones column
     (hs_aug [l, 65]) and the weights with a bias row (W_aug [65, h]).
  2. k @ prev_cache is reassociated as hs_aug @ (Wk_aug @ prev_cache); the
     [65, 512] matrix M_k = Wk_aug @ C is precomputed once.  This removes
     the need for k^T in SBUF (saving a 16 MB transpose + 4.3 GFLOP).
  3. The L2 normalization is folded into per-row scales:
        u0 = hs_aug @ M_k        (un-normalized k0 @ C)
        s  = 1/||k0||_row ;  w = s*v0 - s^2*u0
        out = C + k0^T @ w       (k0 un-normalized!)
     since (D k0)^T (v0 - D u0) with D=diag(s) equals k0^T (s*v0 - s^2*u0).

Matmuls run as float32r (full fp32 storage, fast PE mode).
"""

import numpy as np
from contextlib import ExitStack

import concourse.bass as bass
import concourse.bacc as bacc
import concourse.tile as tile
import concourse.mybir as mybir
from concourse.bass_utils import run_bass_kernel_spmd
from concourse.masks import make_identity

B, L, R, H = 8, 8192, 64, 512
P = 128
NT = L // P            # 64 l-tiles of 128 rows
HC = H // P            # 4 h-chunks of 128
RA = R + 1             # augmented contraction dim (64 + ones row)
RAP = RA + 1           # padded even width for fp32r matmul destinations
F32 = mybir.dt.float32
F32R = mybir.dt.float32r
AF = mybir.ActivationFunctionType
OP = mybir.AluOpType

_cache = {}
QUAD_STATS = False
PIPE = True
PIPE_DEPTH = 8
GRAM = False
MERGED = False
CFG = {"hin": 4, "hsT": 3, "k0": 12, "v0s": 2, "w": 10, "sq": 2, "k0ps": 2, "v0ps": 1, "u0ps": 1}


def _mm(nc, out, lhsT, rhs, **kw):
    assert lhsT.dtype == F32R and rhs.dtype == F32R, (lhsT.dtype, rhs.dtype)
    nc.tensor.matmul(out, lhsT, rhs, **kw)


def _body(tc, out_d, ins, reps=1):
    nc = tc.nc
    hs = ins["hidden_states"]
    cache = ins["prev_cache"]
    kw_d = ins["key_w"]
    kb_d = ins["key_b"]
    vw_d = ins["value_w"]
    vb_d = ins["value_b"]

    with ExitStack() as ctx:
        pool = lambda name, bufs, **kw: ctx.enter_context(
            tc.tile_pool(name=name, bufs=bufs, **kw)
        )
        singles = pool("singles", 1)
        hin_pool = pool("hin", CFG["hin"])
        hsT_pool = pool("hsT", CFG["hsT"])
        k0_pool = pool("k0", CFG["k0"])
        v0s_pool = pool("v0s", CFG["v0s"])
        w_pool = pool("w", CFG["w"])
        sq_pool = pool("sq", CFG["sq"])
        stat_pool = pool("stat", 8)
        out_pool = pool("outp", 1)
        # PSUM: 16 KB/partition = 8 banks total
        acc_ps_pool = pool("acc_ps", 1, space="PSUM")      # 4 banks
        k0_ps_pool = pool("k0_ps", CFG["k0ps"], space="PSUM")
        if CFG.get("vu_shared"):
            vu_ps_pool = pool("vu_ps", 1, space="PSUM")
            v0_ps_pool = u0_ps_pool = vu_ps_pool
        else:
            v0_ps_pool = pool("v0_ps", CFG["v0ps"], space="PSUM")
            u0_ps_pool = pool("u0_ps", CFG["u0ps"], space="PSUM")

        # ---- constants ----
        ident = singles.tile([P, P], F32)
        make_identity(nc, ident)
        ident_r = singles.tile([P, P], F32R)
        nc.scalar.copy(ident_r, ident)
        one = singles.tile([P, 1], F32)
        nc.vector.memset(one, 1.0)
        one3 = singles.tile([P, 4, 1], F32)
        nc.vector.memset(one3, 1.0)

        # prefetch first hs quads (DMA + transpose) before the big cache DMA
        # so PE starts early
        hs_q = hs.rearrange("(q t p) r -> q p t r", p=P, t=4)
        hin_prefetch = {}
        for q in range(2):
            hin = hin_pool.tile([P, 4, RA], F32R, tag="hin")
            nc.sync.dma_start(hin[:, :, :R], hs_q[q])
            nc.scalar.activation(hin[:, :, R : R + 1], one3, AF.Copy)
            hsT_ps = k0_ps_pool.tile([RA, 4, P], F32R, tag="k0ps")
            for t in range(4):
                nc.tensor.transpose(hsT_ps[:, t, :], hin[:, t, :], ident_r)
            hsT = hsT_pool.tile([RA, 4, P], F32R, tag="hsT")
            nc.vector.tensor_copy(hsT, hsT_ps)
            hin_prefetch[q] = (hin, hsT)

        wk_aug = singles.tile([RA, H], F32R)
        nc.gpsimd.dma_start(wk_aug[:R, :], kw_d)
        nc.gpsimd.dma_start(wk_aug[R : R + 1, :], kb_d.unsqueeze(0))
        wv_aug = singles.tile([RA, H], F32R)
        nc.gpsimd.dma_start(wv_aug[:R, :], vw_d)
        nc.gpsimd.dma_start(wv_aug[R : R + 1, :], vb_d.unsqueeze(0))

        c_r = singles.tile([P, HC, H], F32R)
        nc.gpsimd.dma_start(c_r, cache.rearrange("(c p) d -> p c d", p=P))

        # ---- WkT_aug = (Wk_aug)^T  [h, 66] via PE transposes ----
        wkT_ps = k0_ps_pool.tile([P, HC, RAP], F32R, tag="k0ps")
        for c in range(HC):
            nc.tensor.transpose(
                wkT_ps[:, c, :], wk_aug[:, c * P : (c + 1) * P], ident_r[:RA, :RAP]
            )
        wkT = singles.tile([P, HC, RAP], F32R)
        nc.scalar.copy(wkT, wkT_ps)

        # ---- M_k = Wk_aug @ C   [65, 512] ----
        mk_ps = v0_ps_pool.tile([RAP, H], F32, tag="v0ps")
        for c in range(HC):
            _mm(nc, mk_ps, wkT[:, c, :], c_r[:, c, :], start=(c == 0), stop=(c == HC - 1))
        mk = singles.tile([RAP, H], F32R)
        nc.scalar.copy(mk, mk_ps)

        if GRAM:
            # ---- G = Wk_aug @ Wk_aug^T  (for ssq = rowsum(hs_aug * (hs_aug G))) ----
            g_ps = u0_ps_pool.tile([RAP, RAP], F32, tag="v0ps" if CFG.get("vu_shared") else "u0_ps")
            for c in range(HC):
                _mm(nc, g_ps, wkT[:, c, :], wkT[:, c, :], start=(c == 0), stop=(c == HC - 1))
            gmat = singles.tile([RAP, RAP], F32R)
            nc.scalar.copy(gmat, g_ps)
        else:
            gmat = None

        # ---- main loop over 64 l-tiles (in quads sharing a transpose bank) ----
        for rep in range(reps):
            acc = acc_ps_pool.tile([P, HC, H], F32, tag="acc")
            pending = []
            for q in range(NT // 4):
                if rep == 0 and q in hin_prefetch:
                    hin, hsT = hin_prefetch.pop(q)
                else:
                    hin = hin_pool.tile([P, 4, RA], F32R, tag="hin")
                    nc.sync.dma_start(hin[:, :, :R], hs_q[q])
                    nc.scalar.activation(hin[:, :, R : R + 1], one3, AF.Copy)
                    hsT_ps = k0_ps_pool.tile([RA, 4, P], F32R, tag="k0ps")
                    for t in range(4):
                        nc.tensor.transpose(hsT_ps[:, t, :], hin[:, t, :], ident_r)
                    hsT = hsT_pool.tile([RA, 4, P], F32R, tag="hsT")
                    nc.vector.tensor_copy(hsT, hsT_ps)

                # per-quad: row stats (via Gram matrix) + k-projections
                k0s = []
                stats = []
                for t in range(4):
                    if MERGED and not GRAM:
                        break
                    lhs = hsT[:, t, :]
                    k0_ps0 = None
                    ssq = stat_pool.tile([P, 1], F32, tag="ssq")
                    if GRAM:
                        # ssq_l = hs_aug[l] G hs_aug[l]^T = rowsum(hs_aug * (hs_aug @ G))
                        p0_ps = k0_ps_pool.tile([P, RAP], F32, tag="k0ps")
                        _mm(nc, p0_ps, lhs, gmat[:RA, :], start=True, stop=True)
                        sq = sq_pool.tile([P, RA], F32)
                        nc.vector.scalar_tensor_tensor(
                            out=sq, in0=p0_ps[:, :RA], scalar=one, in1=hin[:, t, :],
                            op0=OP.mult, op1=OP.mult, accum_out=ssq,
                        )
                    else:
                        k0_ps0 = k0_ps_pool.tile([P, H], F32, tag="k0ps")
                        _mm(nc, k0_ps0, lhs, wk_aug, start=True, stop=True)
                        k0e = k0_pool.tile([P, H], F32R, tag="k0")
                        nc.scalar.copy(k0e, k0_ps0)
                        sq = sq_pool.tile([P, H], F32, tag="sqbig")
                        nc.vector.scalar_tensor_tensor(
                            out=sq, in0=k0e.bitcast(F32), scalar=one, in1=k0e.bitcast(F32),
                            op0=OP.mult, op1=OP.mult, accum_out=ssq,
                        )
                    nrm = stat_pool.tile([P, 1], F32, tag="nrm")
                    nc.scalar.activation(nrm, ssq, AF.Sqrt)
                    s_ap = stat_pool.tile([P, 1], F32, tag="s")
                    nc.vector.reciprocal(s_ap, nrm)
                    ns2_ap = stat_pool.tile([P, 1], F32, tag="ns2")
                    nc.vector.scalar_tensor_tensor(
                        out=ns2_ap, in0=s_ap, scalar=-1.0, in1=s_ap,
                        op0=OP.mult, op1=OP.mult,
                    )
                    stats.append((s_ap, ns2_ap))

                    if GRAM:
                        k0_ps = k0_ps_pool.tile([P, H], F32, tag="k0ps")
                        _mm(nc, k0_ps, lhs, wk_aug, start=True, stop=True)
                        k0 = k0_pool.tile([P, H], F32R, tag="k0")
                        nc.scalar.copy(k0, k0_ps)
                        k0s.append(k0)
                    else:
                        k0s.append(k0e)

                def emit_step4(k0_, w_, i_):
                    for hc in range(HC):
                        _mm(
                            nc, acc[:, hc, :], k0_[:, hc * P : (hc + 1) * P], w_,
                            start=(i_ == 0), stop=(i_ == NT - 1),
                        )

                for t in range(4):
                    lhs = hsT[:, t, :]
                    i = q * 4 + t
                    if MERGED and not GRAM:
                        k0_ps0 = k0_ps_pool.tile([P, H], F32, tag="k0ps")
                        _mm(nc, k0_ps0, lhs, wk_aug, start=True, stop=True)
                        k0e = k0_pool.tile([P, H], F32R, tag="k0")
                        nc.scalar.copy(k0e, k0_ps0)
                        k0s.append(k0e)
                        ssq = stat_pool.tile([P, 1], F32, tag="ssq")
                        sq = sq_pool.tile([P, H], F32, tag="sqbig")
                        nc.vector.scalar_tensor_tensor(
                            out=sq, in0=k0e.bitcast(F32), scalar=one, in1=k0e.bitcast(F32),
                            op0=OP.mult, op1=OP.mult, accum_out=ssq,
                        )
                        nrm = stat_pool.tile([P, 1], F32, tag="nrm")
                        nc.scalar.activation(nrm, ssq, AF.Sqrt)
                        s_ap = stat_pool.tile([P, 1], F32, tag="s")
                        nc.vector.reciprocal(s_ap, nrm)
                        ns2_ap = stat_pool.tile([P, 1], F32, tag="ns2")
                        nc.vector.scalar_tensor_tensor(
                            out=ns2_ap, in0=s_ap, scalar=-1.0, in1=s_ap,
                            op0=OP.mult, op1=OP.mult,
                        )
                    else:
                        s_ap, ns2_ap = stats[t]
                    v0_ps = v0_ps_pool.tile([P, H], F32, tag="v0ps")
                    _mm(nc, v0_ps, lhs, wv_aug, start=True, stop=True)
                    u0_ps = u0_ps_pool.tile([P, H], F32, tag="v0ps" if CFG.get("vu_shared") else "u0_ps")
                    _mm(nc, u0_ps, lhs, mk[:RA, :], start=True, stop=True)
                    # v0s = s * v0
                    v0s = v0s_pool.tile([P, H], F32)
                    nc.scalar.activation(v0s, v0_ps, AF.Copy, scale=s_ap)
                    # w = s*v0 - s^2*u0 = (u0 * -s^2) + v0s
                    w = w_pool.tile([P, H], F32R)
                    nc.vector.scalar_tensor_tensor(
                        out=w, in0=u0_ps, scalar=ns2_ap, in1=v0s,
                        op0=OP.mult, op1=OP.add,
                    )
                    if PIPE:
                        # software pipeline: step-4 lags so PE never waits on
                        # the v0s->w chain
                        pending.append((k0s[t], w, i))
                        if len(pending) > PIPE_DEPTH:
                            emit_step4(*pending.pop(0))
                    else:
                        emit_step4(k0s[t], w, i)

            while PIPE and pending:
                emit_step4(*pending.pop(0))

            out_sb = out_pool.tile([P, HC, H], F32)
            for hc in range(HC):
                nc.vector.tensor_add(
                    out_sb[:, hc, :], acc[:, hc, :], c_r.bitcast(F32)[:, hc, :]
                )
                nc.sync.dma_start(
                    out_d.rearrange("(c p) d -> p c d", p=P)[:, hc, :], out_sb[:, hc, :]
                )


def _build(reps=1):
    nc = bacc.Bacc("TRN2", target_bir_lowering=False, debug=False, num_devices=B)
    ins = {
        "hidden_states": nc.dram_tensor("hs", [L, R], F32R, kind="ExternalInput").ap(),
        "prev_cache": nc.dram_tensor("cache", [H, H], F32R, kind="ExternalInput").ap(),
        "key_w": nc.dram_tensor("key_w", [R, H], F32R, kind="ExternalInput").ap(),
        "key_b": nc.dram_tensor("key_b", [H], F32R, kind="ExternalInput").ap(),
        "value_w": nc.dram_tensor("value_w", [R, H], F32R, kind="ExternalInput").ap(),
        "value_b": nc.dram_tensor("value_b", [H], F32R, kind="ExternalInput").ap(),
    }
    out_d = nc.dram_tensor("out", [H, H], F32, kind="ExternalOutput").ap()
    with tile.TileContext(nc) as tc:
        _body(tc, out_d, ins, reps=reps)
    nc.compile()
    return nc


def _get_runner():
    """Build (once) a cached jitted shard_map over the bass_exec custom call.

    run_bass_kernel_spmd re-traces and re-compiles per call; this caches the
    executable so repeat calls only pay transfer + execution.
    """
    if "runner" in _cache:
        return _cache["runner"]
    import jax
    from jax.sharding import Mesh, PartitionSpec
    from jax.experimental.shard_map import shard_map
    from concourse.bass2jax import (
        _bass_exec_p,
        partition_id_tensor,
        install_neuronx_cc_hook,
    )

    nc = _build()
    install_neuronx_cc_hook()
    partition_name = nc.partition_id_tensor.name if nc.partition_id_tensor else None
    in_names, out_names, out_avals = [], [], []
    for alloc in nc.m.functions[0].allocations:
        if not isinstance(alloc, mybir.MemoryLocationSet):
            continue
        name = alloc.memorylocations[0].name
        if alloc.kind == "ExternalInput":
            if name != partition_name:
                in_names.append(name)
        elif alloc.kind == "ExternalOutput":
            out_names.append(name)
            out_avals.append(
                jax.core.ShapedArray(tuple(alloc.tensor_shape), mybir.dt.np(alloc.dtype))
            )
    n_params = len(in_names)
    n_outs = len(out_avals)
    all_in_names = list(in_names) + list(out_names)
    if partition_name is not None:
        all_in_names.append(partition_name)

    def _bass_body(*args):
        operands = list(args)
        if partition_name is not None:
            operands.append(partition_id_tensor())
        return tuple(
            _bass_exec_p.bind(
                *operands,
                out_avals=tuple(out_avals),
                in_names=tuple(all_in_names),
                out_names=tuple(out_names),
                lowering_input_output_aliases=(),
                sim_require_finite=True,
                sim_require_nnan=True,
                nc=nc,
            )
        )

    devices = jax.devices()[:B]
    assert len(devices) == B, f"need {B} devices, have {len(jax.devices())}"
    mesh = Mesh(np.asarray(devices), ("core",))
    in_specs = (PartitionSpec("core"),) * (n_params + n_outs)
    out_specs = (PartitionSpec("core"),) * n_outs
    donate = tuple(range(n_params, n_params + n_outs))
    fn = jax.jit(
        shard_map(
            _bass_body, mesh=mesh, in_specs=in_specs, out_specs=out_specs,
            check_rep=False,
        ),
        donate_argnums=donate,
        keep_unused=True,
    )
    import jax.numpy as jnp
    from jax.sharding import NamedSharding

    zero_shardings = [NamedSharding(mesh, PartitionSpec("core"))] * n_outs

    @jax.jit
    def _zeros():
        return tuple(
            jnp.zeros((B * a.shape[0], *a.shape[1:]), a.dtype) for a in out_avals
        )

    zeros_fn = jax.jit(_zeros, out_shardings=tuple(zero_shardings))
    _cache["zeros_fn"] = zeros_fn
    _cache["runner"] = (fn, in_names, out_names, out_avals)
    return _cache["runner"]


def kernel(**inputs) -> np.ndarray:
    hs = np.ascontiguousarray(np.asarray(inputs["hidden_states"], dtype=np.float32))
    pc = np.ascontiguousarray(np.asarray(inputs["prev_cache"], dtype=np.float32))
    kw = np.ascontiguousarray(np.asarray(inputs["key_w"], dtype=np.float32))
    kb = np.ascontiguousarray(np.asarray(inputs["key_b"], dtype=np.float32))
    vw = np.ascontiguousarray(np.asarray(inputs["value_w"], dtype=np.float32))
    vb = np.ascontiguousarray(np.asarray(inputs["value_b"], dtype=np.float32))

    fn, in_names, out_names, out_avals = _get_runner()
    per_core = {
        "hs": hs.reshape(B * L, R),
        "cache": pc.reshape(B * H, H),
        "key_w": np.concatenate([kw] * B, axis=0),
        "key_b": np.concatenate([kb] * B, axis=0),
        "value_w": np.concatenate([vw] * B, axis=0),
        "value_b": np.concatenate([vb] * B, axis=0),
    }
    concat_in = [per_core[n] for n in in_names]
    zeros = _cache["zeros_fn"]()
    out_arrs = fn(*concat_in, *zeros)
    out = np.asarray(out_arrs[out_names.index("out")])
    return out.reshape(B, H, H)

